# revision 10
# baseline (speedup 1.0000x reference)
"""Trainium2 (Bass/Tile) kernel for quantized multi-head attention.

Distributed across 8 NeuronCores: tensor-parallel over heads for the
Q4_0-dequant + QKV projections + RoPE + causal attention, per-batch
AllGather collectives (overlapped with compute), then a column-sharded
output projection (each core dequantizes only its 512 wo out-channels
and produces out[:, shard] for all tokens). All weight transposes ride
the DMA X-bar (zero TensorE transposes); the softmax partition-sum
runs on GpSimd. Host-side work is limited to input marshalling
(sharding, layout transposes of inputs, small derived tables) and
concatenating the per-core output column slices.
"""

import math
from dataclasses import dataclass

import numpy as np

import concourse.bass as bass
import concourse.tile as tile
from concourse import bacc, mybir, bass_isa

BF = mybir.dt.bfloat16
F32 = mybir.dt.float32
I8 = mybir.dt.int8
AOP = mybir.AluOpType
AF = mybir.ActivationFunctionType


@dataclass
class Cfg:
    B: int = 4
    S: int = 1024
    D: int = 4096
    NCORES: int = 8
    SCH: int = 512   # kept for test.py compat (unused)
    QCH: int = 512   # attention q-chunk

    @property
    def T(self):
        return self.B * self.S

    @property
    def H(self):
        return self.D // 128  # total heads (head_dim 128)

    @property
    def H_LOC(self):
        return self.H // self.NCORES

    @property
    def C_SHARD(self):
        return self.H_LOC * 128  # local channels

    @property
    def NGP(self):
        return self.D // 128  # contraction k-tiles / group-pairs per row


def build_program(cfg: Cfg):
    """Build the per-core Bass program. Returns compiled nc."""
    c = cfg
    assert c.S % c.QCH == 0 and c.QCH <= 512

    # raise the stale SBUF cap (224KB phys, ~208 usable per partition)
    import concourse.tile_utils as tile_utils
    tile_utils.max_sbuf_usage = 208 * 1024

    nc = bacc.Bacc("TRN2", target_bir_lowering=False, debug=False,
                   num_devices=c.NCORES)

    OSH = c.C_SHARD  # weight shard out-channels per core (qkv and wo)
    # ---- external I/O ----
    x_d = nc.dram_tensor("x", [c.D, c.T], BF, kind="ExternalInput")  # pre-transposed
    RPO = c.NGP          # packed rows per out-channel
    GPO = 2 * c.NGP      # scale groups per out-channel
    w_q = nc.dram_tensor("wq_w", [OSH * RPO, 64], I8, kind="ExternalInput")
    s_q = nc.dram_tensor("wq_s", [OSH * GPO, 1], BF, kind="ExternalInput")
    w_k = nc.dram_tensor("wk_w", [OSH * RPO, 64], I8, kind="ExternalInput")
    s_k = nc.dram_tensor("wk_s", [OSH * GPO, 1], BF, kind="ExternalInput")
    w_v = nc.dram_tensor("wv_w", [OSH * RPO, 64], I8, kind="ExternalInput")
    s_v = nc.dram_tensor("wv_s", [OSH * GPO, 1], BF, kind="ExternalInput")
    w_o = nc.dram_tensor("wo_w", [OSH * RPO, 64], I8, kind="ExternalInput")
    s_o = nc.dram_tensor("wo_s", [OSH * GPO, 1], BF, kind="ExternalInput")
    # rope tables, replicated over local heads; partition = s % 128
    cos4_d = nc.dram_tensor("cos4", [128, c.S // 128, c.C_SHARD], BF,
                            kind="ExternalInput")
    sins4_d = nc.dram_tensor("sins4", [128, c.S // 128, c.C_SHARD], BF,
                             kind="ExternalInput")
    maskd_d = nc.dram_tensor("maskd", [128, 128], BF, kind="ExternalInput")
    out_d = nc.dram_tensor("out", [c.T, OSH], BF, kind="ExternalOutput")

    # collective buffers, one AllGather per batch
    ag_in = [nc.dram_tensor(f"ag_in{b}", [c.C_SHARD, c.S], BF)
             for b in range(c.B)]
    ag_out = [nc.dram_tensor(f"ag_out{b}", [c.NCORES, c.C_SHARD, c.S], BF)
              for b in range(c.B)]
    # dequantized+transposed wo panel, staged via DRAM during phase 1
    wto_d = nc.dram_tensor("wto", [128, c.NGP, OSH], BF)

    inv_sqrt_d = 1.0 / math.sqrt(128.0)
    ngp = c.NGP
    half = ngp // 2  # 16 scale-group-pairs per xbar transpose slab

    def dequant_ob(sbuf, pw_v, ps_v, orow, write_comb):
        """Dequantize one 128-out-channel block; write_comb(hb, comb) sinks
        each [128, half, 128] bf16 slab (natural layout: partition = oc)."""
        p_nat = sbuf.tile([128, ngp * 64], I8, tag="dq_p", bufs=2)
        nc.sync.dma_start(p_nat[:], pw_v[orow:orow + 128, :])
        s_nat = sbuf.tile([128, ngp * 2], BF, tag="dq_s", bufs=2)
        nc.sync.dma_start(s_nat[:], ps_v[orow:orow + 128, :])
        for hb in range(2):
            g0 = hb * half
            comb = sbuf.tile([128, half, 128], BF, tag="dq_comb", bufs=2)
            # shift-free nibble extract: hi = b & 0xF0 == 16*msb
            # (scales table ships s_even/16 so the 16 cancels)
            msb = sbuf.tile([128, half * 64], I8, tag="dq_m", bufs=2)
            nc.vector.tensor_scalar(
                out=msb[:], in0=p_nat[:, g0 * 64:(g0 + half) * 64],
                scalar1=-16, scalar2=None, op0=AOP.bitwise_and)
            lsb = sbuf.tile([128, half * 64], I8, tag="dq_l", bufs=2)
            nc.vector.tensor_scalar(
                out=lsb[:], in0=p_nat[:, g0 * 64:(g0 + half) * 64],
                scalar1=15, scalar2=None, op0=AOP.bitwise_and)
            nc.vector.tensor_scalar(
                out=lsb[:], in0=lsb[:],
                scalar1=8, scalar2=None, op0=AOP.bitwise_xor)
            nc.vector.tensor_scalar(
                out=lsb[:], in0=lsb[:],
                scalar1=8, scalar2=None, op0=AOP.subtract)
            nc.vector.tensor_tensor(
                out=comb[:, :, 0:64],
                in0=msb[:].rearrange("o (gp f) -> o gp f", f=64),
                in1=s_nat[:, 2 * g0::2][:, :half, None].to_broadcast(
                    [128, half, 64]),
                op=AOP.mult)
            nc.vector.tensor_tensor(
                out=comb[:, :, 64:128],
                in0=lsb[:].rearrange("o (gp f) -> o gp f", f=64),
                in1=s_nat[:, 2 * g0 + 1::2][:, :half, None].to_broadcast(
                    [128, half, 64]),
                op=AOP.mult)
            write_comb(g0, comb)

    def dequant_to_wt(sbuf, wt, pw, ps, nob):
        """Dequantize packed rows into wt tile [128, NGP, 128*nob] using
        X-bar transposes (wt[:, g, ob*128+j] = W[oc=ob*128+j, c=g*128+p])."""
        pw_v = pw.ap().rearrange("(o r) f -> o (r f)", r=ngp)
        ps_v = ps.ap().rearrange("(o g) one -> o (g one)", g=2 * ngp)
        for ob in range(nob):
            def sink(g0, comb, ob=ob):
                nc.sync.dma_start(
                    out=wt[:, g0:g0 + half, ob * 128:(ob + 1) * 128],
                    in_=comb[:], transpose=True)
            dequant_ob(sbuf, pw_v, ps_v, ob * 128, sink)

    def dequant_to_dram(sbuf, wt_dram, pw, ps, nob):
        """Same as dequant_to_wt but staging each transposed slab through a
        small SBUF tile into a DRAM panel (frees SBUF during phase 1)."""
        pw_v = pw.ap().rearrange("(o r) f -> o (r f)", r=ngp)
        ps_v = ps.ap().rearrange("(o g) one -> o (g one)", g=2 * ngp)
        for ob in range(nob):
            def sink(g0, comb, ob=ob):
                wtmp = sbuf.tile([128, half, 128], BF, tag="dq_wt", bufs=1)
                nc.sync.dma_start(out=wtmp[:], in_=comb[:], transpose=True)
                nc.sync.dma_start(
                    out=wt_dram.ap()[:, g0:g0 + half,
                                     ob * 128:(ob + 1) * 128],
                    in_=wtmp[:])
            dequant_ob(sbuf, pw_v, ps_v, ob * 128, sink)

    with tile.TileContext(nc) as tc:
        with tc.tile_pool(name="const", bufs=1) as const, \
             tc.tile_pool(name="sbuf", bufs=2) as sbuf:
            # constants
            cos4 = const.tile([128, c.S // 128, c.C_SHARD], BF)
            nc.sync.dma_start(cos4[:], cos4_d[:])
            sins4 = const.tile([128, c.S // 128, c.C_SHARD], BF)
            nc.sync.dma_start(sins4[:], sins4_d[:])
            maskd = const.tile([128, 128], BF)
            nc.sync.dma_start(maskd[:], maskd_d[:])

            # ============ phase 1: QKV + attention ============
            with tc.tile_pool(name="wt", bufs=1) as wtp, \
                 tc.tile_pool(name="xt", bufs=3) as xtp, \
                 tc.tile_pool(name="kqv", bufs=1) as kqvp, \
                 tc.tile_pool(name="pt", bufs=4) as ptp, \
                 tc.tile_pool(name="ppsum", bufs=3, space="PSUM") as ppsum, \
                 tc.tile_pool(name="spsum", bufs=3, space="PSUM") as spsum, \
                 tc.tile_pool(name="apsum", bufs=2, space="PSUM") as apsum:

                wt_q = wtp.tile([128, c.NGP, OSH], BF, tag="wt_q")
                wt_k = wtp.tile([128, c.NGP, OSH], BF, tag="wt_k")
                wt_v = wtp.tile([128, c.NGP, OSH], BF, tag="wt_v")
                dequant_to_wt(sbuf, wt_q, w_q, s_q, OSH // 128)
                dequant_to_wt(sbuf, wt_k, w_k, s_k, OSH // 128)
                dequant_to_wt(sbuf, wt_v, w_v, s_v, OSH // 128)

                def project(b, ts, mat, wt_m, kt_b, qt_b, v_b):
                    tt0 = b * c.S + ts * 128
                    st0 = ts * 128
                    xt_ts = xtp.tile([128, c.NGP, 128], BF, tag="xt")
                    nc.sync.dma_start(
                        xt_ts[:],
                        x_d.ap().rearrange(
                            "(g p) t -> p g t", p=128)[:, :, tt0:tt0 + 128])
                    ps = ppsum.tile([128, OSH], F32, tag="proj")
                    for gp in range(c.NGP):
                        nc.tensor.matmul(
                            ps[:],
                            lhsT=xt_ts[:, gp, :],
                            rhs=wt_m[:, gp, :],
                            start=(gp == 0),
                            stop=(gp == c.NGP - 1))
                    if mat == "v":
                        nc.scalar.copy(out=v_b[:, ts, :], in_=ps[:])
                        return
                    # rope: roped = ps*cos4 + swaphalf(ps)*sins4
                    roped = sbuf.tile([128, c.C_SHARD], BF,
                                      tag="roped", bufs=3)
                    tmp = sbuf.tile([128, c.C_SHARD], BF,
                                    tag="ropetmp", bufs=3)
                    p3 = ps[:].rearrange("p (h d) -> p h d", d=128)
                    t3 = tmp[:].rearrange("p (h d) -> p h d", d=128)
                    c3 = cos4[:, ts, :].rearrange("p (h d) -> p h d", d=128)
                    s3 = sins4[:, ts, :].rearrange("p (h d) -> p h d", d=128)
                    nc.vector.tensor_tensor(
                        out=t3[:, :, 0:64], in0=p3[:, :, 64:128],
                        in1=s3[:, :, 0:64], op=AOP.mult)
                    nc.vector.tensor_tensor(
                        out=t3[:, :, 64:128], in0=p3[:, :, 0:64],
                        in1=s3[:, :, 64:128], op=AOP.mult)
                    nc.vector.tensor_tensor(
                        out=roped[:], in0=ps[:], in1=cos4[:, ts, :],
                        op=AOP.mult)
                    nc.vector.tensor_tensor(
                        out=roped[:], in0=roped[:], in1=tmp[:],
                        op=AOP.add)
                    dst = qt_b if mat == "q" else kt_b
                    # X-bar transpose per head: dst[d, h, st0+s] = roped[s, h*128+d]
                    nc.sync.dma_start(
                        out=dst[:, :, st0:st0 + 128],
                        in_=roped[:], transpose=True)

                for b in range(c.B):
                    # per-batch K/Q transposed ([d, s] per head) and V natural
                    kt_b = kqvp.tile([128, c.H_LOC, c.S], BF, tag="kt_b")
                    qt_b = kqvp.tile([128, c.H_LOC, c.S], BF, tag="qt_b")
                    v_b = kqvp.tile([128, c.S // 128, c.C_SHARD], BF,
                                    tag="v_b")
                    if b == 0:
                        # mat-major: Q matmuls start as soon as wt_q is
                        # ready, overlapping wt_k/wt_v dequant
                        for mat, wt_m in (("q", wt_q), ("k", wt_k),
                                          ("v", wt_v)):
                            for ts in range(c.S // 128):
                                project(b, ts, mat, wt_m, kt_b, qt_b, v_b)
                    else:
                        for ts in range(c.S // 128):
                            for mat, wt_m in (("q", wt_q), ("k", wt_k),
                                              ("v", wt_v)):
                                project(b, ts, mat, wt_m, kt_b, qt_b, v_b)

                    # ---- attention for batch b ----
                    for h in range(c.H_LOC):
                        for qc in range(c.S // c.QCH):
                            q0 = qc * c.QCH
                            kmax = (q0 + c.QCH) // 128
                            at = apsum.tile([128, c.QCH], F32, tag="at")
                            psum_tree = sbuf.tile([128, c.QCH], F32,
                                                  tag="ptree", bufs=2)
                            for ki in range(kmax):
                                off = max(0, 128 * ki - q0)
                                stp = spsum.tile([128, c.QCH], F32, tag="sc")
                                nc.tensor.matmul(
                                    stp[:, off:], lhsT=kt_b[:, h, ki * 128:(ki + 1) * 128],
                                    rhs=qt_b[:, h, q0 + off:q0 + c.QCH],
                                    start=True, stop=True)
                                if 128 * ki >= q0:
                                    nc.vector.tensor_tensor(
                                        out=stp[:, off:off + 128],
                                        in0=stp[:, off:off + 128],
                                        in1=maskd[:], op=AOP.add)
                                pt = ptp.tile([128, c.QCH], BF, tag="pt")
                                nc.scalar.activation(
                                    out=pt[:, off:], in_=stp[:, off:],
                                    func=AF.Exp, scale=inv_sqrt_d)
                                # accumulate sum-over-k partials on DVE
                                if ki == 0:
                                    nc.vector.tensor_copy(
                                        out=psum_tree[:], in_=pt[:])
                                else:
                                    nc.vector.tensor_tensor(
                                        out=psum_tree[:, off:],
                                        in0=psum_tree[:, off:],
                                        in1=pt[:, off:], op=AOP.add)
                                nc.tensor.matmul(
                                    at[:, off:],
                                    lhsT=v_b[:, ki, h * 128:(h + 1) * 128],
                                    rhs=pt[:, off:],
                                    start=(ki == 0), stop=(ki == kmax - 1))
                            # z = sum over k-partitions, replicated to all
                            zfull = sbuf.tile([128, c.QCH], F32, tag="zf",
                                              bufs=2)
                            nc.gpsimd.partition_all_reduce(
                                zfull[:], psum_tree[:], channels=128,
                                reduce_op=bass_isa.ReduceOp.add)
                            rz = sbuf.tile([128, c.QCH], F32, tag="rz",
                                           bufs=2)
                            nc.vector.reciprocal_approx_fast(rz[:], zfull[:])
                            ao = sbuf.tile([128, c.QCH], BF, tag="ao")
                            nc.vector.tensor_tensor(
                                out=ao[:], in0=at[:], in1=rz[:], op=AOP.mult)
                            nc.sync.dma_start(
                                out=ag_in[b][h * 128:(h + 1) * 128,
                                             q0:q0 + c.QCH],
                                in_=ao[:])
                    # per-batch collective, overlapped with later batches
                    nc.gpsimd.collective_compute(
                        "AllGather", AOP.bypass,
                        replica_groups=[list(range(c.NCORES))],
                        ins=[ag_in[b].ap().opt()],
                        outs=[ag_out[b].ap().opt()],
                    )
                    if b == 1:
                        # wo dequant during phase-1 DVE slack, staged to DRAM
                        dequant_to_dram(sbuf, wto_d, w_o, s_o, OSH // 128)

            # ==== phase 2: output projection (wo column-sharded) ====
            with tc.tile_pool(name="gath", bufs=3) as gathp, \
                 tc.tile_pool(name="wop", bufs=1) as wopp, \
                 tc.tile_pool(name="wpsum", bufs=2, space="PSUM") as wpsum:
                panel = wopp.tile([128, c.NGP, OSH], BF, tag="wop")
                nc.sync.dma_start(panel[:], wto_d[:])
                for tch in range(c.T // 512):
                    b, scq = tch // 2, tch % 2
                    gch = gathp.tile([128, c.NGP, 512], BF, tag="gch")
                    nc.sync.dma_start(
                        gch[:],
                        ag_out[b].ap().rearrange(
                            "s (g p) t -> p (s g) t",
                            p=128)[:, :, scq * 512:(scq + 1) * 512])
                    for tb in range(4):
                        ops = wpsum.tile([128, OSH], F32, tag="wo")
                        for ct in range(c.NGP):
                            nc.tensor.matmul(
                                ops[:], lhsT=gch[:, ct, tb * 128:(tb + 1) * 128],
                                rhs=panel[:, ct, :],
                                start=(ct == 0), stop=(ct == c.NGP - 1))
                        osb = sbuf.tile([128, OSH], BF, tag="osb", bufs=3)
                        nc.scalar.copy(out=osb[:], in_=ops[:])
                        t0 = tch * 512 + tb * 128
                        nc.sync.dma_start(
                            out=out_d[t0:t0 + 128, :], in_=osb[:])

    nc.compile()
    return nc


# ---------------- host-side input prep ----------------

def prep_core_inputs(cfg: Cfg, x, cos_half, sin_half, mask,
                     wq_w, wq_s, wk_w, wk_s, wv_w, wv_s, wo_w, wo_s):
    """Build in_maps (list of dicts, one per core) from full inputs."""
    import ml_dtypes
    c = cfg
    bf16 = ml_dtypes.bfloat16
    HD2 = 64

    x2 = np.ascontiguousarray(
        np.asarray(x).reshape(c.T, c.D).T)  # ship transposed [D, T]

    # rope tables [128, S//128, C_SHARD]
    ch = np.asarray(cos_half, np.float32)  # [S, 64]
    sh = np.asarray(sin_half, np.float32)
    cos = np.concatenate([ch, ch], axis=1).astype(bf16).astype(np.float32)  # [S,128]
    sin = np.concatenate([sh, sh], axis=1).astype(bf16).astype(np.float32)
    sins = sin.copy()
    sins[:, :HD2] = -sin[:, :HD2]
    cos4 = np.tile(cos[:, None, :], (1, c.H_LOC, 1)).reshape(c.S, c.C_SHARD)
    sins4 = np.tile(sins[:, None, :], (1, c.H_LOC, 1)).reshape(c.S, c.C_SHARD)
    # partition = s % 128, ssub = s // 128
    cos4 = np.ascontiguousarray(
        cos4.reshape(c.S // 128, 128, c.C_SHARD).transpose(1, 0, 2)).astype(bf16)
    sins4 = np.ascontiguousarray(
        sins4.reshape(c.S // 128, 128, c.C_SHARD).transpose(1, 0, 2)).astype(bf16)

    # diagonal mask block: maskd[k, q] from input mask[q, k] (first 128 block)
    m = np.asarray(mask, np.float32)[:128, :128]
    maskd = np.maximum(m.T, -1e30).astype(bf16)

    OSH = c.C_SHARD

    def dq_scales(ps):
        # [N*GPO, 1] -> even groups (msb) divided by 16 (exact in bf16)
        a = np.asarray(ps).astype(np.float32).reshape(-1, 2)
        a[:, 0] /= 16.0
        return np.ascontiguousarray(a.reshape(-1, 1)).astype(bf16)

    in_maps = []
    for core in range(c.NCORES):
        RPO = c.NGP
        r0 = core * OSH * RPO
        g0 = core * OSH * 2 * RPO
        in_maps.append({
            "x": x2.astype(bf16, copy=False),
            "wq_w": np.ascontiguousarray(np.asarray(wq_w)[r0:r0 + OSH * RPO]),
            "wq_s": dq_scales(np.asarray(wq_s)[g0:g0 + OSH * 2 * RPO]),
            "wk_w": np.ascontiguousarray(np.asarray(wk_w)[r0:r0 + OSH * RPO]),
            "wk_s": dq_scales(np.asarray(wk_s)[g0:g0 + OSH * 2 * RPO]),
            "wv_w": np.ascontiguousarray(np.asarray(wv_w)[r0:r0 + OSH * RPO]),
            "wv_s": dq_scales(np.asarray(wv_s)[g0:g0 + OSH * 2 * RPO]),
            "wo_w": np.ascontiguousarray(np.asarray(wo_w)[r0:r0 + OSH * RPO]),
            "wo_s": dq_scales(np.asarray(wo_s)[g0:g0 + OSH * 2 * RPO]),
            "cos4": cos4,
            "sins4": sins4,
            "maskd": maskd,
        })
    return in_maps


def unshard_output(cfg: Cfg, results):
    """results: list per core of {"out": [T, C_SHARD]}; core r's columns
    are out-channels [512r, 512(r+1))."""
    c = cfg
    full = np.concatenate(
        [np.asarray(results[r]["out"]) for r in range(c.NCORES)], axis=1)
    return full.reshape(c.B, c.S, c.D)


# ======================================================================
# Self-contained kernel entry point.
# Accepts FULL (unsharded) inputs as produced by setup_inputs() and
# returns the FULL output [B, S, D] (bfloat16), matching reference().
# Sharding: tensor-parallel over heads for QKV/attention, per-batch
# AllGather, column-sharded output projection; host concatenates the
# per-core output column slices.
# ======================================================================

_CACHE = {}


def _get_program(cfg):
    key = (cfg.B, cfg.S, cfg.D, cfg.NCORES, cfg.SCH, cfg.QCH)
    if key not in _CACHE:
        _CACHE[key] = build_program(cfg)
    return _CACHE[key]


def kernel(x, start_pos=0, cos_half=None, sin_half=None, mask=None,
           wq_w=None, wq_s=None, wk_w=None, wk_s=None,
           wv_w=None, wv_s=None, wo_w=None, wo_s=None,
           cache_k_w=None, cache_k_s=None, cache_v_w=None, cache_v_s=None,
           **_unused):
    from concourse.bass_utils import run_bass_kernel_spmd

    assert int(start_pos) == 0, "kernel specialised for start_pos == 0"
    x = np.asarray(x)
    B, S, D = x.shape
    cfg = Cfg(B=B, S=S, D=D, NCORES=8, SCH=512, QCH=512)
    # start_pos==0 with S==MAX_S, B==MAX_B: the quantized KV cache is fully
    # overwritten before use, so cache_* inputs cannot affect the output.
    in_maps = prep_core_inputs(cfg, x, cos_half, sin_half, mask,
                               wq_w, wq_s, wk_w, wk_s, wv_w, wv_s,
                               wo_w, wo_s)
    nc = _get_program(cfg)
    res = run_bass_kernel_spmd(nc, in_maps, core_ids=list(range(cfg.NCORES)))
    out = unshard_output(cfg, res.results)
    import ml_dtypes
    return out.astype(ml_dtypes.bfloat16, copy=False)
